# revision 6
# baseline (speedup 1.0000x reference)
"""Differential cross-attention kernel for Trainium2, 8-core data-parallel.

Per core (one batch element b):
  qT = (Wq.T/16) @ geneT            [E, NG-chunk]   (fp32r matmuls)
  kT = Wk.T @ subT                  [E, NS]
  v  = sub @ Wv.T                   [NS, E]
  S_i = q_i k_i^T                   [128m, NS] in PSUM  (i = head 1, 2)
  P_i = exp(S_i), d_i = rowsum      (ACT, accum_out)
  diff = P1/d1 - lam*P2/d2          (DVE)  -> HBM output + PE-transpose
  O = diff @ v                      (via transposed diff blocks)
  out = RMSNorm(O) @ (w*(1-l0)*Wo.T)

Inputs are staged on host: gene/substructure transposed per batch, weights
pre-transposed & pre-scaled, lambda computed on host (tiny dot products).
"""
import math

import numpy as np

import concourse.bass as bass
import concourse.mybir as mybir
import concourse.tile as tile
from concourse import bacc
from concourse import bass_utils
from concourse.masks import make_identity

N_CORES = 8
B, NG, NS, E = 8, 4096, 1024, 512
H = E // 2                     # 256, per-head dim
LAMBDA_INIT = 0.8 - 0.6 * math.exp(-0.3 * 0.0)   # depth 0 -> 0.2
RMS_EPS = 1e-5
P = 128                        # partitions
KI = E // P                    # 4 e_in tiles
EO = E // P                    # 4 e_out tiles
NB = NS // P                   # 8 kv tiles
CHUNK = 512                    # m tokens per chunk
NCH = NG // CHUNK              # 8 chunks
JT = CHUNK // P                # 4 m-subtiles per chunk

F32 = mybir.dt.float32
F32R = mybir.dt.float32r
AF = mybir.ActivationFunctionType
ALU = mybir.AluOpType


def build_kernel():
    nc = bacc.Bacc("TRN2", target_bir_lowering=False, debug=False,
                   num_devices=N_CORES)
    geneT = nc.dram_tensor("geneT", [E, NG], F32R, kind="ExternalInput").ap()
    subT = nc.dram_tensor("subT", [E, NS], F32R, kind="ExternalInput").ap()
    wqT = nc.dram_tensor("wqT", [E, E], F32R, kind="ExternalInput").ap()
    wkT = nc.dram_tensor("wkT", [E, E], F32R, kind="ExternalInput").ap()
    wvT = nc.dram_tensor("wvT", [E, E], F32R, kind="ExternalInput").ap()
    woT = nc.dram_tensor("woT", [E, E], F32R, kind="ExternalInput").ap()
    lam = nc.dram_tensor("lam", [P, 1], F32, kind="ExternalInput").ap()
    out_d = nc.dram_tensor("out", [NG, E], F32, kind="ExternalOutput").ap()
    diff_d = nc.dram_tensor("diff", [NG, NS], F32, kind="ExternalOutput").ap()

    with tile.TileContext(nc) as tc:
        emit(tc, geneT, subT, wqT, wkT, wvT, woT, lam, out_d, diff_d)
    nc.compile()
    return nc


def emit(tc, geneT, subT, wqT, wkT, wvT, woT, lam, out_d, diff_d):
    nc = tc.nc
    from contextlib import ExitStack
    with ExitStack() as ctx:
        consts = ctx.enter_context(tc.tile_pool(name="consts", bufs=1))
        kvp = ctx.enter_context(tc.tile_pool(name="kvp", bufs=1))
        gpool = ctx.enter_context(tc.tile_pool(name="gpool", bufs=2))
        qpool = ctx.enter_context(tc.tile_pool(name="qpool", bufs=2))
        ppool = ctx.enter_context(tc.tile_pool(name="ppool", bufs=2))
        dfpool = ctx.enter_context(tc.tile_pool(name="dfpool", bufs=3))
        dTpool = ctx.enter_context(tc.tile_pool(name="dTpool", bufs=2))
        sqpool = ctx.enter_context(tc.tile_pool(name="sqpool", bufs=2))
        nopool = ctx.enter_context(tc.tile_pool(name="nopool", bufs=2))
        ypool = ctx.enter_context(tc.tile_pool(name="ypool", bufs=2))
        dpool = ctx.enter_context(tc.tile_pool(name="dpool", bufs=24))
        # PSUM: 8 banks total = ps_s 3x[128,1024] (6) + ps_u 2x[128,512] (2)
        ps_s = ctx.enter_context(tc.tile_pool(name="ps_s", bufs=3, space="PSUM"))
        ps_u = ctx.enter_context(tc.tile_pool(name="ps_u", bufs=2, space="PSUM"))

        # ---- constants -------------------------------------------------
        w_sb = {}
        for name, src in (("wq", wqT), ("wk", wkT), ("wv", wvT), ("wo", woT)):
            t = consts.tile([P, KI, E], F32R, tag=name)
            nc.sync.dma_start(out=t[:], in_=src.rearrange("(ki p) e -> p ki e", p=P))
            w_sb[name] = t
        ident = consts.tile([P, P], F32, tag="ident")
        make_identity(nc, ident[:])
        lam_sb = consts.tile([P, 1], F32, tag="lam")
        nc.sync.dma_start(out=lam_sb[:], in_=lam)
        eps_sb = consts.tile([P, 1], F32, tag="eps")
        nc.vector.memset(eps_sb[:], RMS_EPS)

        # ---- kv setup: kT [E, NS] and v [NS, E] ------------------------
        subT_sb = kvp.tile([P, KI, NS], F32R, tag="subT")
        nc.sync.dma_start(out=subT_sb[:],
                          in_=subT.rearrange("(ki p) n -> p ki n", p=P))
        kT_sb = kvp.tile([P, EO, NS], F32R, tag="kT")
        v_sb = kvp.tile([P, NB, E], F32R, tag="v")

        for eo in range(EO):
            for nch in range(NS // 512):
                pk = ps_u.tile([P, 512], F32, tag="u")
                for ki in range(KI):
                    nc.tensor.matmul(
                        pk[:],
                        w_sb["wk"][:, ki, eo * P:(eo + 1) * P],
                        subT_sb[:, ki, nch * 512:(nch + 1) * 512],
                        start=(ki == 0), stop=(ki == KI - 1))
                nc.scalar.copy(kT_sb[:, eo, nch * 512:(nch + 1) * 512], pk[:])

        for nb in range(NB):
            pv = ps_u.tile([P, 512], F32, tag="u")
            for ki in range(KI):
                nc.tensor.matmul(
                    pv[:],
                    subT_sb[:, ki, nb * P:(nb + 1) * P],
                    w_sb["wv"][:, ki, :],
                    start=(ki == 0), stop=(ki == KI - 1))
            nc.scalar.copy(v_sb[:, nb, :], pv[:])

        # ---- main loop over m-chunks ----------------------------------
        for c in range(NCH):
            gT = gpool.tile([P, KI, CHUNK], F32R, tag="gT")
            nc.sync.dma_start(
                out=gT[:],
                in_=geneT[:, c * CHUNK:(c + 1) * CHUNK].rearrange(
                    "(ki p) m -> p ki m", p=P))
            qT = qpool.tile([P, EO, CHUNK], F32R, tag="qT")
            for eo in range(EO):
                pq = ps_u.tile([P, 512], F32, tag="u")
                for ki in range(KI):
                    nc.tensor.matmul(
                        pq[:],
                        w_sb["wq"][:, ki, eo * P:(eo + 1) * P],
                        gT[:, ki, :],
                        start=(ki == 0), stop=(ki == KI - 1))
                nc.scalar.copy(qT[:, eo, :], pq[:])

            for j in range(JT):
                m0 = c * CHUNK + j * P
                # S1/S2 logits [128, NS] in PSUM
                s1 = ps_s.tile([P, NS], F32, tag="s")
                s2 = ps_s.tile([P, NS], F32, tag="s")
                for hk in range(2):
                    for nch in range(NS // 512):
                        nc.tensor.matmul(
                            s1[:, nch * 512:(nch + 1) * 512],
                            qT[:, hk, j * P:(j + 1) * P],
                            kT_sb[:, hk, nch * 512:(nch + 1) * 512],
                            start=(hk == 0), stop=(hk == 1))
                for hk in range(2):
                    for nch in range(NS // 512):
                        nc.tensor.matmul(
                            s2[:, nch * 512:(nch + 1) * 512],
                            qT[:, 2 + hk, j * P:(j + 1) * P],
                            kT_sb[:, 2 + hk, nch * 512:(nch + 1) * 512],
                            start=(hk == 0), stop=(hk == 1))

                # softmax numerators + row sums
                p1 = ppool.tile([P, NS], F32, tag="p1")
                d1 = dpool.tile([P, 1], F32, tag="d")
                nc.scalar.activation(out=p1[:], in_=s1[:], func=AF.Exp,
                                     accum_out=d1[:])
                p2 = ppool.tile([P, NS], F32, tag="p2")
                d2 = dpool.tile([P, 1], F32, tag="d")
                nc.scalar.activation(out=p2[:], in_=s2[:], func=AF.Exp,
                                     accum_out=d2[:])
                r1 = dpool.tile([P, 1], F32, tag="d")
                nc.vector.reciprocal(r1[:], d1[:])
                r2 = dpool.tile([P, 1], F32, tag="d")
                nc.vector.reciprocal(r2[:], d2[:])
                # p2 <- (p2 * 1/d2) * lam in place; diff = p1*r1 - p2
                nc.vector.tensor_scalar(out=p2[:], in0=p2[:], scalar1=r2[:],
                                        scalar2=lam_sb[:], op0=ALU.mult,
                                        op1=ALU.mult)
                diff = dfpool.tile([P, NS], F32, tag="diff")
                nc.vector.scalar_tensor_tensor(
                    out=diff[:], in0=p1[:], scalar=r1[:], in1=p2[:],
                    op0=ALU.mult, op1=ALU.subtract)
                nc.sync.dma_start(out=diff_d[m0:m0 + P, :], in_=diff[:])

                # transpose diff -> diffT (PE) in two bank-sized halves
                dT = dTpool.tile([P, NS], F32R, tag="dT")
                for half in range(2):
                    tt = ps_u.tile([P, 512], F32, tag="u")
                    for nbh in range(4):
                        nb = half * 4 + nbh
                        nc.tensor.transpose(tt[:, nbh * P:(nbh + 1) * P],
                                            diff[:, nb * P:(nb + 1) * P],
                                            ident[:])
                    nc.vector.tensor_copy(dT[:, half * 512:(half + 1) * 512],
                                          tt[:])

                # O = diff @ v  [128, E]
                po = ps_u.tile([P, E], F32, tag="u")
                for nb in range(NB):
                    nc.tensor.matmul(
                        po[:],
                        dT[:, nb * P:(nb + 1) * P],
                        v_sb[:, nb, :],
                        start=(nb == 0), stop=(nb == NB - 1))

                # RMS norm: rstd = exp(-0.5*ln(mean(O^2)+eps))
                # (Ln/Exp/Square/Copy share one ACT table set -> no reloads)
                sq = sqpool.tile([P, E], F32, tag="sq")
                ssq = dpool.tile([P, 1], F32, tag="d")
                nc.scalar.activation(out=sq[:], in_=po[:], func=AF.Square,
                                     accum_out=ssq[:])
                lt = dpool.tile([P, 1], F32, tag="d")
                nc.scalar.activation(out=lt[:], in_=ssq[:], func=AF.Ln,
                                     scale=1.0 / E, bias=eps_sb[:])
                rstd = dpool.tile([P, 1], F32, tag="d")
                nc.scalar.activation(out=rstd[:], in_=lt[:], func=AF.Exp,
                                     scale=-0.5)
                no = nopool.tile([P, E], F32, tag="no")
                nc.vector.tensor_scalar_mul(no[:], po[:], rstd[:])

                # transpose normed O, project with Wo
                nt = ps_u.tile([P, E], F32, tag="u")
                for eb in range(EO):
                    nc.tensor.transpose(nt[:, eb * P:(eb + 1) * P],
                                        no[:, eb * P:(eb + 1) * P], ident[:])
                noT = nopool.tile([P, E], F32R, tag="noT")
                nc.vector.tensor_copy(noT[:], nt[:])
                py = ps_u.tile([P, E], F32, tag="u")
                for eb in range(EO):
                    nc.tensor.matmul(
                        py[:],
                        noT[:, eb * P:(eb + 1) * P],
                        w_sb["wo"][:, eb, :],
                        start=(eb == 0), stop=(eb == EO - 1))
                y = ypool.tile([P, E], F32, tag="y")
                nc.vector.tensor_copy(y[:], py[:])
                nc.sync.dma_start(out=out_d[m0:m0 + P, :], in_=y[:])


# ---------------------------------------------------------------------------
_NC = None


def get_nc():
    global _NC
    if _NC is None:
        _NC = build_kernel()
    return _NC


def stage_inputs(gene, substructure, Wq, Wk, Wv, Wo,
                 lambda_q1, lambda_k1, lambda_q2, lambda_k2, rms_weight):
    gene = np.asarray(gene, np.float32)
    substructure = np.asarray(substructure, np.float32)
    scaling = H ** -0.5
    lam_full = (math.exp(float(np.sum(np.asarray(lambda_q1, np.float64) *
                                      np.asarray(lambda_k1, np.float64))))
                - math.exp(float(np.sum(np.asarray(lambda_q2, np.float64) *
                                        np.asarray(lambda_k2, np.float64))))
                + LAMBDA_INIT)
    wqT = np.ascontiguousarray(np.asarray(Wq, np.float32).T * scaling)
    wkT = np.ascontiguousarray(np.asarray(Wk, np.float32).T)
    wvT = np.ascontiguousarray(np.asarray(Wv, np.float32).T)
    woT = np.ascontiguousarray(
        np.asarray(rms_weight, np.float32)[:, None]
        * np.asarray(Wo, np.float32).T * (1.0 - LAMBDA_INIT))
    lam_tile = np.full((P, 1), lam_full, np.float32)
    in_maps = []
    for b in range(N_CORES):
        in_maps.append({
            "geneT": np.ascontiguousarray(gene[b].T),
            "subT": np.ascontiguousarray(substructure[b].T),
            "wqT": wqT, "wkT": wkT, "wvT": wvT, "woT": woT,
            "lam": lam_tile,
        })
    return in_maps


def run(in_maps, trace=False, **kw):
    nc = get_nc()
    last_err = None
    for attempt in range(3):
        try:
            return bass_utils.run_bass_kernel_spmd(
                nc, in_maps, core_ids=list(range(N_CORES)), trace=trace, **kw)
        except Exception as e:  # transient device errors on first touch
            last_err = e
    raise last_err


def kernel(**inputs):
    in_maps = stage_inputs(**inputs)
    res = run(in_maps, trace=False)
    out = np.stack([res.results[b]["out"] for b in range(N_CORES)])
    diff = np.stack([res.results[b]["diff"] for b in range(N_CORES)])
    return out, diff


# revision 17
# speedup vs baseline: 1.2582x; 1.2582x over previous
"""Differential cross-attention kernel for Trainium2, 8-core data-parallel.

Per core (one batch element b):
  qT = (Wq.T/16) @ geneT            [E, NG-chunk]   (fp32r matmuls)
  kT = Wk.T @ subT                  [E, NS]
  v  = sub @ Wv.T                   [NS, E]
  S_i = q_i k_i^T                   [128m, NS] in PSUM  (i = head 1, 2)
  P_i = exp(S_i), d_i = rowsum      (ACT, accum_out)
  diff = P1/d1 - lam*P2/d2          (DVE)  -> HBM output + PE-transpose
  O = diff @ v                      (via transposed diff blocks)
  out = RMSNorm(O) @ (w*(1-l0)*Wo.T)

Inputs are staged on host: gene/substructure transposed per batch, weights
pre-transposed & pre-scaled, lambda computed on host (tiny dot products).
"""
import math

import numpy as np

import concourse.bass as bass
import concourse.mybir as mybir
import concourse.tile as tile
from concourse import bacc
from concourse import bass_utils
from concourse.masks import make_identity

N_CORES = 8
B, NG, NS, E = 8, 4096, 1024, 512
H = E // 2                     # 256, per-head dim
LAMBDA_INIT = 0.8 - 0.6 * math.exp(-0.3 * 0.0)   # depth 0 -> 0.2
RMS_EPS = 1e-5
P = 128                        # partitions
KI = E // P                    # 4 e_in tiles
EO = E // P                    # 4 e_out tiles
NB = NS // P                   # 8 kv tiles
CHUNK = 512                    # m tokens per chunk
NCH = NG // CHUNK              # 8 chunks
JT = CHUNK // P                # 4 m-subtiles per chunk

F32 = mybir.dt.float32
F32R = mybir.dt.float32r
AF = mybir.ActivationFunctionType
ALU = mybir.AluOpType


def build_kernel():
    nc = bacc.Bacc("TRN2", target_bir_lowering=False, debug=False,
                   num_devices=N_CORES)
    geneT = nc.dram_tensor("geneT", [E, NG], F32R, kind="ExternalInput").ap()
    subT = nc.dram_tensor("subT", [E, NS], F32R, kind="ExternalInput").ap()
    wqT = nc.dram_tensor("wqT", [E, E], F32R, kind="ExternalInput").ap()
    wkT = nc.dram_tensor("wkT", [E, E], F32R, kind="ExternalInput").ap()
    wvT = nc.dram_tensor("wvT", [E, E], F32R, kind="ExternalInput").ap()
    woT = nc.dram_tensor("woT", [E, E], F32R, kind="ExternalInput").ap()
    lam = nc.dram_tensor("lam", [P, 1], F32, kind="ExternalInput").ap()
    out_d = nc.dram_tensor("out", [NG, E], F32, kind="ExternalOutput").ap()
    diff_d = nc.dram_tensor("diff", [NG, NS], F32, kind="ExternalOutput").ap()

    with tile.TileContext(nc) as tc:
        emit(tc, geneT, subT, wqT, wkT, wvT, woT, lam, out_d, diff_d)
    nc.compile()
    return nc


def emit(tc, geneT, subT, wqT, wkT, wvT, woT, lam, out_d, diff_d):
    nc = tc.nc
    from contextlib import ExitStack
    with ExitStack() as ctx:
        consts = ctx.enter_context(tc.tile_pool(name="consts", bufs=1))
        kvp = ctx.enter_context(tc.tile_pool(name="kvp", bufs=1))
        gpool = ctx.enter_context(tc.tile_pool(name="gpool", bufs=2))
        qpool = ctx.enter_context(tc.tile_pool(name="qpool", bufs=2))
        ppool = ctx.enter_context(tc.tile_pool(name="ppool", bufs=2))
        dfpool = ctx.enter_context(tc.tile_pool(name="dfpool", bufs=3))
        dTpool = ctx.enter_context(tc.tile_pool(name="dTpool", bufs=2))
        sqpool = ctx.enter_context(tc.tile_pool(name="sqpool", bufs=2))
        nopool = ctx.enter_context(tc.tile_pool(name="nopool", bufs=2))
        ypool = ctx.enter_context(tc.tile_pool(name="ypool", bufs=2))
        dpool = ctx.enter_context(tc.tile_pool(name="dpool", bufs=64))
        # PSUM: 8 banks total = ps_s 2x[128,1024] (4) + ps_u 4x[128,512] (4)
        ps_s = ctx.enter_context(tc.tile_pool(name="ps_s", bufs=2, space="PSUM"))
        ps_u = ctx.enter_context(tc.tile_pool(name="ps_u", bufs=4, space="PSUM"))

        # ---- constants -------------------------------------------------
        w_sb = {}
        for name, src in (("wq", wqT), ("wk", wkT), ("wv", wvT), ("wo", woT)):
            t = consts.tile([P, KI, E], F32R, tag=name)
            nc.sync.dma_start(out=t[:], in_=src.rearrange("(ki p) e -> p ki e", p=P))
            w_sb[name] = t
        ident = consts.tile([P, P], F32, tag="ident")
        make_identity(nc, ident[:])
        lam_sb = consts.tile([P, 1], F32, tag="lam")
        nc.sync.dma_start(out=lam_sb[:], in_=lam)

        # ---- kv setup: kT [E, NS] and v [NS, E] ------------------------
        subT_sb = kvp.tile([P, KI, NS], F32R, tag="subT")
        nc.sync.dma_start(out=subT_sb[:],
                          in_=subT.rearrange("(ki p) n -> p ki n", p=P))
        kT_sb = kvp.tile([P, EO, NS], F32R, tag="kT")
        v_sb = kvp.tile([P, NB, E], F32R, tag="v")

        for eo in range(EO):
            for nch in range(NS // 512):
                pk = ps_u.tile([P, 512], F32, tag="u")
                for ki in range(KI):
                    nc.tensor.matmul(
                        pk[:],
                        w_sb["wk"][:, ki, eo * P:(eo + 1) * P],
                        subT_sb[:, ki, nch * 512:(nch + 1) * 512],
                        start=(ki == 0), stop=(ki == KI - 1))
                nc.scalar.copy(kT_sb[:, eo, nch * 512:(nch + 1) * 512], pk[:])

        for nb in range(NB):
            pv = ps_u.tile([P, 512], F32, tag="u")
            for ki in range(KI):
                nc.tensor.matmul(
                    pv[:],
                    subT_sb[:, ki, nb * P:(nb + 1) * P],
                    w_sb["wv"][:, ki, :],
                    start=(ki == 0), stop=(ki == KI - 1))
            nc.scalar.copy(v_sb[:, nb, :], pv[:])

        # ---- main loop over m-chunks ----------------------------------
        for c in range(NCH):
            gT = gpool.tile([P, KI, CHUNK], F32R, tag="gT")
            nc.sync.dma_start(
                out=gT[:],
                in_=geneT[:, c * CHUNK:(c + 1) * CHUNK].rearrange(
                    "(ki p) m -> p ki m", p=P))
            qT = qpool.tile([P, EO, CHUNK], F32R, tag="qT")
            for eo in range(EO):
                pq = ps_u.tile([P, 512], F32, tag="u")
                for ki in range(KI):
                    nc.tensor.matmul(
                        pq[:],
                        w_sb["wq"][:, ki, eo * P:(eo + 1) * P],
                        gT[:, ki, :],
                        start=(ki == 0), stop=(ki == KI - 1))
                nc.scalar.copy(qT[:, eo, :], pq[:])

            for j in range(JT):
                m0 = c * CHUNK + j * P
                # S1/S2 logits [128, NS] in PSUM
                s1 = ps_s.tile([P, NS], F32, tag="s")
                s2 = ps_s.tile([P, NS], F32, tag="s")
                for hk in range(2):
                    for nch in range(NS // 512):
                        nc.tensor.matmul(
                            s1[:, nch * 512:(nch + 1) * 512],
                            qT[:, hk, j * P:(j + 1) * P],
                            kT_sb[:, hk, nch * 512:(nch + 1) * 512],
                            start=(hk == 0), stop=(hk == 1))
                for hk in range(2):
                    for nch in range(NS // 512):
                        nc.tensor.matmul(
                            s2[:, nch * 512:(nch + 1) * 512],
                            qT[:, 2 + hk, j * P:(j + 1) * P],
                            kT_sb[:, 2 + hk, nch * 512:(nch + 1) * 512],
                            start=(hk == 0), stop=(hk == 1))

                # softmax numerators + row sums
                p1 = ppool.tile([P, NS], F32, tag="p1")
                d1 = dpool.tile([P, 1], F32, tag="d")
                nc.scalar.activation(out=p1[:], in_=s1[:], func=AF.Exp,
                                     accum_out=d1[:])
                p2 = ppool.tile([P, NS], F32, tag="p2")
                d2 = dpool.tile([P, 1], F32, tag="d")
                nc.scalar.activation(out=p2[:], in_=s2[:], func=AF.Exp,
                                     accum_out=d2[:])
                r1 = dpool.tile([P, 1], F32, tag="d")
                nc.vector.reciprocal(r1[:], d1[:])
                r2 = dpool.tile([P, 1], F32, tag="d")
                nc.vector.reciprocal(r2[:], d2[:])
                # p2 <- (p2 * 1/d2) * lam in place; diff = p1*r1 - p2
                nc.vector.tensor_scalar(out=p2[:], in0=p2[:], scalar1=r2[:],
                                        scalar2=lam_sb[:], op0=ALU.mult,
                                        op1=ALU.mult)
                diff = dfpool.tile([P, NS], F32, tag="diff")
                nc.vector.scalar_tensor_tensor(
                    out=diff[:], in0=p1[:], scalar=r1[:], in1=p2[:],
                    op0=ALU.mult, op1=ALU.subtract)
                nc.sync.dma_start(out=diff_d[m0:m0 + P, :], in_=diff[:])

                # transpose diff -> diffT (PE) in two bank-sized halves
                dT = dTpool.tile([P, NS], F32R, tag="dT")
                for half in range(2):
                    tt = ps_u.tile([P, 512], F32, tag="u")
                    for nbh in range(4):
                        nb = half * 4 + nbh
                        nc.tensor.transpose(tt[:, nbh * P:(nbh + 1) * P],
                                            diff[:, nb * P:(nb + 1) * P],
                                            ident[:])
                    nc.vector.tensor_copy(dT[:, half * 512:(half + 1) * 512],
                                          tt[:])

                # O = diff @ v  [128, E]  (unnormalized; rstd applied at the
                # very end since the row-scale commutes with the Wo matmul)
                po = ps_u.tile([P, E], F32, tag="u")
                for nb in range(NB):
                    nc.tensor.matmul(
                        po[:],
                        dT[:, nb * P:(nb + 1) * P],
                        v_sb[:, nb, :],
                        start=(nb == 0), stop=(nb == NB - 1))

                # RMS statistic: ssq via ACT Square+accum (same table set as
                # Exp -> no ACT table reloads anywhere in the kernel).
                o_sb = nopool.tile([P, E], F32, tag="no")
                nc.scalar.copy(o_sb[:], po[:])
                sq = sqpool.tile([P, E], F32, tag="sq")
                ssq = dpool.tile([P, 1], F32, tag="d")
                nc.scalar.activation(out=sq[:], in_=po[:], func=AF.Square,
                                     accum_out=ssq[:])
                # rstd = 1/sqrt(ssq/E + eps) via Quake bit-trick + 2 Newton
                # iterations on DVE (avoids the ACT Sqrt table thrash).
                t_ssq = dpool.tile([P, 1], F32, tag="d")
                nc.vector.tensor_scalar(out=t_ssq[:], in0=ssq[:],
                                        scalar1=1.0 / E, scalar2=RMS_EPS,
                                        op0=ALU.mult, op1=ALU.add)
                I32 = mybir.dt.int32
                ihalf = dpool.tile([P, 1], F32, tag="d")
                nc.vector.tensor_scalar(
                    out=ihalf[:].bitcast(I32), in0=t_ssq[:].bitcast(I32),
                    scalar1=1, scalar2=None, op0=ALU.arith_shift_right)
                inot = dpool.tile([P, 1], F32, tag="d")
                nc.vector.tensor_scalar(
                    out=inot[:].bitcast(I32), in0=ihalf[:].bitcast(I32),
                    scalar1=-1, scalar2=None, op0=ALU.bitwise_xor)
                yq = dpool.tile([P, 1], F32, tag="d")
                nc.vector.tensor_scalar(
                    out=yq[:].bitcast(I32), in0=inot[:].bitcast(I32),
                    scalar1=0x5f3759df + 1, scalar2=None, op0=ALU.add)
                rstd = yq
                for _ in range(2):
                    y2 = dpool.tile([P, 1], F32, tag="d")
                    nc.vector.tensor_mul(y2[:], rstd[:], rstd[:])
                    w = dpool.tile([P, 1], F32, tag="d")
                    nc.vector.tensor_scalar(out=w[:], in0=y2[:],
                                            scalar1=t_ssq[:], scalar2=-0.5,
                                            op0=ALU.mult, op1=ALU.mult)
                    yn = dpool.tile([P, 1], F32, tag="d")
                    nc.vector.scalar_tensor_tensor(
                        out=yn[:], in0=w[:], scalar=1.5, in1=rstd[:],
                        op0=ALU.add, op1=ALU.mult)
                    rstd = yn

                # transpose O, project with Wo, scale by rstd at the end
                nt = ps_u.tile([P, E], F32, tag="u")
                for eb in range(EO):
                    nc.tensor.transpose(nt[:, eb * P:(eb + 1) * P],
                                        o_sb[:, eb * P:(eb + 1) * P], ident[:])
                noT = nopool.tile([P, E], F32R, tag="noT")
                nc.scalar.copy(noT[:], nt[:])
                py = ps_u.tile([P, E], F32, tag="u")
                for eb in range(EO):
                    nc.tensor.matmul(
                        py[:],
                        noT[:, eb * P:(eb + 1) * P],
                        w_sb["wo"][:, eb, :],
                        start=(eb == 0), stop=(eb == EO - 1))
                y = ypool.tile([P, E], F32, tag="y")
                nc.scalar.activation(out=y[:], in_=py[:], func=AF.Copy,
                                     scale=rstd[:])
                nc.sync.dma_start(out=out_d[m0:m0 + P, :], in_=y[:])


# ---------------------------------------------------------------------------
_NC = None


def get_nc():
    global _NC
    if _NC is None:
        _NC = build_kernel()
    return _NC


def stage_inputs(gene, substructure, Wq, Wk, Wv, Wo,
                 lambda_q1, lambda_k1, lambda_q2, lambda_k2, rms_weight):
    gene = np.asarray(gene, np.float32)
    substructure = np.asarray(substructure, np.float32)
    scaling = H ** -0.5
    lam_full = (math.exp(float(np.sum(np.asarray(lambda_q1, np.float64) *
                                      np.asarray(lambda_k1, np.float64))))
                - math.exp(float(np.sum(np.asarray(lambda_q2, np.float64) *
                                        np.asarray(lambda_k2, np.float64))))
                + LAMBDA_INIT)
    wqT = np.ascontiguousarray(np.asarray(Wq, np.float32).T * scaling)
    wkT = np.ascontiguousarray(np.asarray(Wk, np.float32).T)
    wvT = np.ascontiguousarray(np.asarray(Wv, np.float32).T)
    woT = np.ascontiguousarray(
        np.asarray(rms_weight, np.float32)[:, None]
        * np.asarray(Wo, np.float32).T * (1.0 - LAMBDA_INIT))
    lam_tile = np.full((P, 1), lam_full, np.float32)
    in_maps = []
    for b in range(N_CORES):
        in_maps.append({
            "geneT": np.ascontiguousarray(gene[b].T),
            "subT": np.ascontiguousarray(substructure[b].T),
            "wqT": wqT, "wkT": wkT, "wvT": wvT, "woT": woT,
            "lam": lam_tile,
        })
    return in_maps


def run(in_maps, trace=False, **kw):
    nc = get_nc()
    last_err = None
    for attempt in range(3):
        try:
            return bass_utils.run_bass_kernel_spmd(
                nc, in_maps, core_ids=list(range(N_CORES)), trace=trace, **kw)
        except Exception as e:  # transient device errors on first touch
            last_err = e
    raise last_err


def kernel(**inputs):
    in_maps = stage_inputs(**inputs)
    res = run(in_maps, trace=False)
    out = np.stack([res.results[b]["out"] for b in range(N_CORES)])
    diff = np.stack([res.results[b]["diff"] for b in range(N_CORES)])
    return out, diff


# revision 22
# speedup vs baseline: 1.2678x; 1.0076x over previous
"""Differential cross-attention kernel for Trainium2, 8-core data-parallel.

Per core (one batch element b):
  qT = (Wq.T/16) @ geneT            [E, NG-chunk]   (fp32r matmuls)
  kT = Wk.T @ subT                  [E, NS]
  v  = sub @ Wv.T                   [NS, E]
  S_i = q_i k_i^T                   [128m, NS] in PSUM  (i = head 1, 2)
  P_i = exp(S_i), d_i = rowsum      (ACT, accum_out)
  diff = P1/d1 - lam*P2/d2          (DVE)  -> HBM output + PE-transpose
  O = diff @ v                      (via transposed diff blocks)
  out = RMSNorm(O) @ (w*(1-l0)*Wo.T)

Inputs are staged on host: gene/substructure transposed per batch, weights
pre-transposed & pre-scaled, lambda computed on host (tiny dot products).
"""
import math

import numpy as np

import concourse.bass as bass
import concourse.mybir as mybir
import concourse.tile as tile
from concourse import bacc
from concourse import bass_utils
from concourse.masks import make_identity

N_CORES = 8
B, NG, NS, E = 8, 4096, 1024, 512
H = E // 2                     # 256, per-head dim
LAMBDA_INIT = 0.8 - 0.6 * math.exp(-0.3 * 0.0)   # depth 0 -> 0.2
RMS_EPS = 1e-5
P = 128                        # partitions
KI = E // P                    # 4 e_in tiles
EO = E // P                    # 4 e_out tiles
NB = NS // P                   # 8 kv tiles
CHUNK = 512                    # m tokens per chunk
NCH = NG // CHUNK              # 8 chunks
JT = CHUNK // P                # 4 m-subtiles per chunk

F32 = mybir.dt.float32
F32R = mybir.dt.float32r
AF = mybir.ActivationFunctionType
ALU = mybir.AluOpType


def build_kernel():
    nc = bacc.Bacc("TRN2", target_bir_lowering=False, debug=False,
                   num_devices=N_CORES)
    geneT = nc.dram_tensor("geneT", [E, NG], F32R, kind="ExternalInput").ap()
    subT = nc.dram_tensor("subT", [E, NS], F32R, kind="ExternalInput").ap()
    wqT = nc.dram_tensor("wqT", [E, E], F32R, kind="ExternalInput").ap()
    wkT = nc.dram_tensor("wkT", [E, E], F32R, kind="ExternalInput").ap()
    wvT = nc.dram_tensor("wvT", [E, E], F32R, kind="ExternalInput").ap()
    woT = nc.dram_tensor("woT", [E, E], F32R, kind="ExternalInput").ap()
    lam = nc.dram_tensor("lam", [P, 1], F32, kind="ExternalInput").ap()
    out_d = nc.dram_tensor("out", [NG, E], F32, kind="ExternalOutput").ap()
    diff_d = nc.dram_tensor("diff", [NG, NS], F32R, kind="ExternalOutput").ap()

    with tile.TileContext(nc) as tc:
        emit(tc, geneT, subT, wqT, wkT, wvT, woT, lam, out_d, diff_d)
    nc.compile()
    return nc


def emit(tc, geneT, subT, wqT, wkT, wvT, woT, lam, out_d, diff_d):
    nc = tc.nc
    from contextlib import ExitStack
    with ExitStack() as ctx:
        consts = ctx.enter_context(tc.tile_pool(name="consts", bufs=1))
        kvp = ctx.enter_context(tc.tile_pool(name="kvp", bufs=1))
        gpool = ctx.enter_context(tc.tile_pool(name="gpool", bufs=2))
        qpool = ctx.enter_context(tc.tile_pool(name="qpool", bufs=2))
        ppool = ctx.enter_context(tc.tile_pool(name="ppool", bufs=2))
        dfpool = ctx.enter_context(tc.tile_pool(name="dfpool", bufs=3))
        dTpool = ctx.enter_context(tc.tile_pool(name="dTpool", bufs=2))
        sqpool = ctx.enter_context(tc.tile_pool(name="sqpool", bufs=2))
        nopool = ctx.enter_context(tc.tile_pool(name="nopool", bufs=2))
        ypool = ctx.enter_context(tc.tile_pool(name="ypool", bufs=2))
        dpool = ctx.enter_context(tc.tile_pool(name="dpool", bufs=64))
        # PSUM: 8 banks total = ps_s 2x[128,1024] (4) + ps_u 4x[128,512] (4)
        ps_s = ctx.enter_context(tc.tile_pool(name="ps_s", bufs=2, space="PSUM"))
        ps_u = ctx.enter_context(tc.tile_pool(name="ps_u", bufs=4, space="PSUM"))

        # ---- constants -------------------------------------------------
        w_sb = {}
        for name, src in (("wq", wqT), ("wk", wkT), ("wv", wvT), ("wo", woT)):
            t = consts.tile([P, KI, E], F32R, tag=name)
            nc.sync.dma_start(out=t[:], in_=src.rearrange("(ki p) e -> p ki e", p=P))
            w_sb[name] = t
        ident_f = consts.tile([P, P], F32, tag="ident_f")
        make_identity(nc, ident_f[:])
        ident = consts.tile([P, P], F32R, tag="ident")
        nc.vector.tensor_copy(ident[:], ident_f[:])
        lam_sb = consts.tile([P, 1], F32, tag="lam")
        nc.sync.dma_start(out=lam_sb[:], in_=lam)

        # ---- kv setup: kT [E, NS] and v [NS, E] ------------------------
        subT_sb = kvp.tile([P, KI, NS], F32R, tag="subT")
        nc.sync.dma_start(out=subT_sb[:],
                          in_=subT.rearrange("(ki p) n -> p ki n", p=P))
        kT_sb = kvp.tile([P, EO, NS], F32R, tag="kT")
        v_sb = kvp.tile([P, NB, E], F32R, tag="v")

        for eo in range(EO):
            for nch in range(NS // 512):
                pk = ps_u.tile([P, 512], F32, tag="u")
                for ki in range(KI):
                    nc.tensor.matmul(
                        pk[:],
                        w_sb["wk"][:, ki, eo * P:(eo + 1) * P],
                        subT_sb[:, ki, nch * 512:(nch + 1) * 512],
                        start=(ki == 0), stop=(ki == KI - 1))
                nc.scalar.copy(kT_sb[:, eo, nch * 512:(nch + 1) * 512], pk[:])

        for nb in range(NB):
            pv = ps_u.tile([P, 512], F32, tag="u")
            for ki in range(KI):
                nc.tensor.matmul(
                    pv[:],
                    subT_sb[:, ki, nb * P:(nb + 1) * P],
                    w_sb["wv"][:, ki, :],
                    start=(ki == 0), stop=(ki == KI - 1))
            nc.scalar.copy(v_sb[:, nb, :], pv[:])

        # ---- main loop over m-chunks ----------------------------------
        def produce_qT(c):
            gT = gpool.tile([P, KI, CHUNK], F32R, tag="gT")
            nc.sync.dma_start(
                out=gT[:],
                in_=geneT[:, c * CHUNK:(c + 1) * CHUNK].rearrange(
                    "(ki p) m -> p ki m", p=P))
            qT = qpool.tile([P, EO, CHUNK], F32R, tag="qT")
            for eo in range(EO):
                pq = ps_u.tile([P, 512], F32, tag="u")
                for ki in range(KI):
                    nc.tensor.matmul(
                        pq[:],
                        w_sb["wq"][:, ki, eo * P:(eo + 1) * P],
                        gT[:, ki, :],
                        start=(ki == 0), stop=(ki == KI - 1))
                nc.scalar.copy(qT[:, eo, :], pq[:])
            return qT

        qT = produce_qT(0)
        for c in range(NCH):
            for j in range(JT):
                if j == 2 and c + 1 < NCH:
                    # hoist next chunk's q projection into this chunk so the
                    # gene DMA + proj + copies hide under attention compute
                    next_qT = produce_qT(c + 1)
                m0 = c * CHUNK + j * P
                # S1/S2 logits [128, NS] in PSUM
                s1 = ps_s.tile([P, NS], F32, tag="s")
                s2 = ps_s.tile([P, NS], F32, tag="s")
                for hk in range(2):
                    for nch in range(NS // 512):
                        nc.tensor.matmul(
                            s1[:, nch * 512:(nch + 1) * 512],
                            qT[:, hk, j * P:(j + 1) * P],
                            kT_sb[:, hk, nch * 512:(nch + 1) * 512],
                            start=(hk == 0), stop=(hk == 1))
                for hk in range(2):
                    for nch in range(NS // 512):
                        nc.tensor.matmul(
                            s2[:, nch * 512:(nch + 1) * 512],
                            qT[:, 2 + hk, j * P:(j + 1) * P],
                            kT_sb[:, 2 + hk, nch * 512:(nch + 1) * 512],
                            start=(hk == 0), stop=(hk == 1))

                # softmax numerators + row sums
                p1 = ppool.tile([P, NS], F32, tag="p1")
                d1 = dpool.tile([P, 1], F32, tag="d")
                nc.scalar.activation(out=p1[:], in_=s1[:], func=AF.Exp,
                                     accum_out=d1[:])
                p2 = ppool.tile([P, NS], F32, tag="p2")
                d2 = dpool.tile([P, 1], F32, tag="d")
                nc.scalar.activation(out=p2[:], in_=s2[:], func=AF.Exp,
                                     accum_out=d2[:])
                r1 = dpool.tile([P, 1], F32, tag="d")
                nc.vector.reciprocal(r1[:], d1[:])
                r2 = dpool.tile([P, 1], F32, tag="d")
                nc.vector.reciprocal(r2[:], d2[:])
                # p2 <- (p2 * 1/d2) * lam in place; diff = p1*r1 - p2
                nc.vector.tensor_scalar(out=p2[:], in0=p2[:], scalar1=r2[:],
                                        scalar2=lam_sb[:], op0=ALU.mult,
                                        op1=ALU.mult)
                diff = dfpool.tile([P, NS], F32R, tag="diff")
                nc.vector.scalar_tensor_tensor(
                    out=diff[:], in0=p1[:], scalar=r1[:], in1=p2[:],
                    op0=ALU.mult, op1=ALU.subtract)
                nc.sync.dma_start(out=diff_d[m0:m0 + P, :], in_=diff[:])

                # transpose diff -> diffT (PE) in two bank-sized halves
                dT = dTpool.tile([P, NS], F32R, tag="dT")
                for half in range(2):
                    tt = ps_u.tile([P, 512], F32R, tag="u")
                    for nbh in range(4):
                        nb = half * 4 + nbh
                        nc.tensor.transpose(tt[:, nbh * P:(nbh + 1) * P],
                                            diff[:, nb * P:(nb + 1) * P],
                                            ident[:])
                    nc.vector.tensor_copy(dT[:, half * 512:(half + 1) * 512],
                                          tt[:])

                # O = diff @ v  [128, E]  (unnormalized; rstd applied at the
                # very end since the row-scale commutes with the Wo matmul)
                po = ps_u.tile([P, E], F32, tag="u")
                for nb in range(NB):
                    nc.tensor.matmul(
                        po[:],
                        dT[:, nb * P:(nb + 1) * P],
                        v_sb[:, nb, :],
                        start=(nb == 0), stop=(nb == NB - 1))

                # RMS statistic: t = eps + sum(O*O)/E in one DVE pass
                o_sb = nopool.tile([P, E], F32R, tag="no")
                nc.scalar.copy(o_sb[:], po[:])
                sq = sqpool.tile([P, E], F32, tag="sq")
                ssq = dpool.tile([P, 1], F32, tag="d")
                nc.vector.scalar_tensor_tensor(
                    out=sq[:], in0=o_sb[:], scalar=1.0, in1=o_sb[:],
                    op0=ALU.mult, op1=ALU.mult, accum_out=ssq[:])
                t_ssq = dpool.tile([P, 1], F32, tag="d")
                nc.vector.tensor_scalar(out=t_ssq[:], in0=ssq[:],
                                        scalar1=1.0 / E, scalar2=RMS_EPS,
                                        op0=ALU.mult, op1=ALU.add)
                # rstd = 1/sqrt(t) via Quake bit-trick + 2 Newton iterations
                # on DVE (avoids the ACT Sqrt table thrash).
                I32 = mybir.dt.int32
                ihalf = dpool.tile([P, 1], F32, tag="d")
                nc.vector.tensor_scalar(
                    out=ihalf[:].bitcast(I32), in0=t_ssq[:].bitcast(I32),
                    scalar1=1, scalar2=None, op0=ALU.arith_shift_right)
                inot = dpool.tile([P, 1], F32, tag="d")
                nc.vector.tensor_scalar(
                    out=inot[:].bitcast(I32), in0=ihalf[:].bitcast(I32),
                    scalar1=-1, scalar2=None, op0=ALU.bitwise_xor)
                yq = dpool.tile([P, 1], F32, tag="d")
                nc.vector.tensor_scalar(
                    out=yq[:].bitcast(I32), in0=inot[:].bitcast(I32),
                    scalar1=0x5f3759df + 1, scalar2=None, op0=ALU.add)
                rstd = yq
                for _ in range(2):
                    y2 = dpool.tile([P, 1], F32, tag="d")
                    nc.vector.tensor_mul(y2[:], rstd[:], rstd[:])
                    w = dpool.tile([P, 1], F32, tag="d")
                    nc.vector.tensor_scalar(out=w[:], in0=y2[:],
                                            scalar1=t_ssq[:], scalar2=-0.5,
                                            op0=ALU.mult, op1=ALU.mult)
                    yn = dpool.tile([P, 1], F32, tag="d")
                    nc.vector.scalar_tensor_tensor(
                        out=yn[:], in0=w[:], scalar=1.5, in1=rstd[:],
                        op0=ALU.add, op1=ALU.mult)
                    rstd = yn

                # transpose O, project with Wo, scale by rstd at the end
                nt = ps_u.tile([P, E], F32R, tag="u")
                for eb in range(EO):
                    nc.tensor.transpose(nt[:, eb * P:(eb + 1) * P],
                                        o_sb[:, eb * P:(eb + 1) * P], ident[:])
                noT = nopool.tile([P, E], F32R, tag="noT")
                nc.scalar.copy(noT[:], nt[:])
                py = ps_u.tile([P, E], F32, tag="u")
                for eb in range(EO):
                    nc.tensor.matmul(
                        py[:],
                        noT[:, eb * P:(eb + 1) * P],
                        w_sb["wo"][:, eb, :],
                        start=(eb == 0), stop=(eb == EO - 1))
                y = ypool.tile([P, E], F32, tag="y")
                nc.scalar.activation(out=y[:], in_=py[:], func=AF.Copy,
                                     scale=rstd[:])
                nc.sync.dma_start(out=out_d[m0:m0 + P, :], in_=y[:])
            if c + 1 < NCH:
                qT = next_qT


# ---------------------------------------------------------------------------
_NC = None


def get_nc():
    global _NC
    if _NC is None:
        _NC = build_kernel()
    return _NC


def stage_inputs(gene, substructure, Wq, Wk, Wv, Wo,
                 lambda_q1, lambda_k1, lambda_q2, lambda_k2, rms_weight):
    gene = np.asarray(gene, np.float32)
    substructure = np.asarray(substructure, np.float32)
    scaling = H ** -0.5
    lam_full = (math.exp(float(np.sum(np.asarray(lambda_q1, np.float64) *
                                      np.asarray(lambda_k1, np.float64))))
                - math.exp(float(np.sum(np.asarray(lambda_q2, np.float64) *
                                        np.asarray(lambda_k2, np.float64))))
                + LAMBDA_INIT)
    wqT = np.ascontiguousarray(np.asarray(Wq, np.float32).T * scaling)
    wkT = np.ascontiguousarray(np.asarray(Wk, np.float32).T)
    wvT = np.ascontiguousarray(np.asarray(Wv, np.float32).T)
    woT = np.ascontiguousarray(
        np.asarray(rms_weight, np.float32)[:, None]
        * np.asarray(Wo, np.float32).T * (1.0 - LAMBDA_INIT))
    lam_tile = np.full((P, 1), lam_full, np.float32)
    in_maps = []
    for b in range(N_CORES):
        in_maps.append({
            "geneT": np.ascontiguousarray(gene[b].T),
            "subT": np.ascontiguousarray(substructure[b].T),
            "wqT": wqT, "wkT": wkT, "wvT": wvT, "woT": woT,
            "lam": lam_tile,
        })
    return in_maps


def run(in_maps, trace=False, **kw):
    nc = get_nc()
    last_err = None
    for attempt in range(3):
        try:
            return bass_utils.run_bass_kernel_spmd(
                nc, in_maps, core_ids=list(range(N_CORES)), trace=trace, **kw)
        except Exception as e:  # transient device errors on first touch
            last_err = e
    raise last_err


def kernel(**inputs):
    in_maps = stage_inputs(**inputs)
    res = run(in_maps, trace=False)
    out = np.stack([res.results[b]["out"] for b in range(N_CORES)])
    diff = np.stack([res.results[b]["diff"] for b in range(N_CORES)])
    return out, diff


# revision 23
# speedup vs baseline: 1.9543x; 1.5416x over previous
"""Differential cross-attention kernel for Trainium2, 8-core data-parallel.

Per core (one batch element b):
  qT = (Wq.T/16) @ geneT            [E, NG-chunk]   (fp32r matmuls)
  kT = Wk.T @ subT                  [E, NS]
  v  = sub @ Wv.T                   [NS, E]
  S_i = q_i k_i^T                   [128m, NS] in PSUM  (i = head 1, 2)
  P_i = exp(S_i), d_i = rowsum      (ACT, accum_out)
  diff = P1/d1 - lam*P2/d2          (DVE)  -> HBM output + PE-transpose
  O = diff @ v                      (via transposed diff blocks)
  out = RMSNorm(O) @ (w*(1-l0)*Wo.T)

Inputs are staged on host: gene/substructure transposed per batch, weights
pre-transposed & pre-scaled, lambda computed on host (tiny dot products).
"""
import math

import numpy as np

import concourse.bass as bass
import concourse.mybir as mybir
import concourse.tile as tile
from concourse import bacc
from concourse import bass_utils
from concourse.masks import make_identity

N_CORES = 8
B, NG, NS, E = 8, 4096, 1024, 512
H = E // 2                     # 256, per-head dim
LAMBDA_INIT = 0.8 - 0.6 * math.exp(-0.3 * 0.0)   # depth 0 -> 0.2
RMS_EPS = 1e-5
P = 128                        # partitions
KI = E // P                    # 4 e_in tiles
EO = E // P                    # 4 e_out tiles
NB = NS // P                   # 8 kv tiles
CHUNK = 512                    # m tokens per chunk
NCH = NG // CHUNK              # 8 chunks
JT = CHUNK // P                # 4 m-subtiles per chunk

F32 = mybir.dt.float32
F32R = mybir.dt.float32r
AF = mybir.ActivationFunctionType
ALU = mybir.AluOpType


def build_kernel():
    nc = bacc.Bacc("TRN2", target_bir_lowering=False, debug=False,
                   num_devices=N_CORES)
    geneT = nc.dram_tensor("geneT", [E, NG], F32R, kind="ExternalInput").ap()
    subT = nc.dram_tensor("subT", [E, NS], F32R, kind="ExternalInput").ap()
    wqT = nc.dram_tensor("wqT", [E, E], F32R, kind="ExternalInput").ap()
    wkT = nc.dram_tensor("wkT", [E, E], F32R, kind="ExternalInput").ap()
    wvT = nc.dram_tensor("wvT", [E, E], F32R, kind="ExternalInput").ap()
    woT = nc.dram_tensor("woT", [E, E], F32R, kind="ExternalInput").ap()
    lam = nc.dram_tensor("lam", [P, 1], F32, kind="ExternalInput").ap()
    out_d = nc.dram_tensor("out", [NG, E], F32, kind="ExternalOutput").ap()
    diff_d = nc.dram_tensor("diff", [NG, NS], F32R, kind="ExternalOutput").ap()

    with tile.TileContext(nc) as tc:
        emit(tc, geneT, subT, wqT, wkT, wvT, woT, lam, out_d, diff_d)
    nc.compile()
    return nc


def emit(tc, geneT, subT, wqT, wkT, wvT, woT, lam, out_d, diff_d):
    nc = tc.nc
    from contextlib import ExitStack
    with ExitStack() as ctx:
        consts = ctx.enter_context(tc.tile_pool(name="consts", bufs=1))
        kvp = ctx.enter_context(tc.tile_pool(name="kvp", bufs=1))
        gpool = ctx.enter_context(tc.tile_pool(name="gpool", bufs=2))
        qpool = ctx.enter_context(tc.tile_pool(name="qpool", bufs=2))
        ppool = ctx.enter_context(tc.tile_pool(name="ppool", bufs=2))
        dfpool = ctx.enter_context(tc.tile_pool(name="dfpool", bufs=3))
        dTpool = ctx.enter_context(tc.tile_pool(name="dTpool", bufs=2))
        sqpool = ctx.enter_context(tc.tile_pool(name="sqpool", bufs=2))
        nopool = ctx.enter_context(tc.tile_pool(name="nopool", bufs=2))
        ypool = ctx.enter_context(tc.tile_pool(name="ypool", bufs=2))
        dpool = ctx.enter_context(tc.tile_pool(name="dpool", bufs=64))
        # PSUM: 8 banks total = ps_s 2x[128,1024] (4) + ps_u 4x[128,512] (4)
        ps_s = ctx.enter_context(tc.tile_pool(name="ps_s", bufs=2, space="PSUM"))
        ps_u = ctx.enter_context(tc.tile_pool(name="ps_u", bufs=4, space="PSUM"))

        # ---- constants -------------------------------------------------
        w_sb = {}
        for name, src in (("wq", wqT), ("wk", wkT), ("wv", wvT), ("wo", woT)):
            t = consts.tile([P, KI, E], F32R, tag=name)
            nc.sync.dma_start(out=t[:], in_=src.rearrange("(ki p) e -> p ki e", p=P))
            w_sb[name] = t
        ident_f = consts.tile([P, P], F32, tag="ident_f")
        make_identity(nc, ident_f[:])
        ident = consts.tile([P, P], F32R, tag="ident")
        nc.vector.tensor_copy(ident[:], ident_f[:])
        lam_sb = consts.tile([P, 1], F32, tag="lam")
        nc.sync.dma_start(out=lam_sb[:], in_=lam)

        # ---- kv setup: kT [E, NS] and v [NS, E] ------------------------
        subT_sb = kvp.tile([P, KI, NS], F32R, tag="subT")
        nc.sync.dma_start(out=subT_sb[:],
                          in_=subT.rearrange("(ki p) n -> p ki n", p=P))
        kT_sb = kvp.tile([P, EO, NS], F32R, tag="kT")
        v_sb = kvp.tile([P, NB, E], F32R, tag="v")

        for eo in range(EO):
            for nch in range(NS // 512):
                pk = ps_u.tile([P, 512], F32, tag="u")
                for ki in range(KI):
                    nc.tensor.matmul(
                        pk[:],
                        w_sb["wk"][:, ki, eo * P:(eo + 1) * P],
                        subT_sb[:, ki, nch * 512:(nch + 1) * 512],
                        start=(ki == 0), stop=(ki == KI - 1))
                nc.scalar.copy(kT_sb[:, eo, nch * 512:(nch + 1) * 512], pk[:])

        for nb in range(NB):
            pv = ps_u.tile([P, 512], F32, tag="u")
            for ki in range(KI):
                nc.tensor.matmul(
                    pv[:],
                    subT_sb[:, ki, nb * P:(nb + 1) * P],
                    w_sb["wv"][:, ki, :],
                    start=(ki == 0), stop=(ki == KI - 1))
            nc.scalar.copy(v_sb[:, nb, :], pv[:])

        # ---- main loop over m-chunks ----------------------------------
        def produce_qT(c):
            gT = gpool.tile([P, KI, CHUNK], F32R, tag="gT")
            nc.sync.dma_start(
                out=gT[:],
                in_=geneT[:, c * CHUNK:(c + 1) * CHUNK].rearrange(
                    "(ki p) m -> p ki m", p=P))
            qT = qpool.tile([P, EO, CHUNK], F32R, tag="qT")
            for eo in range(EO):
                pq = ps_u.tile([P, 512], F32, tag="u")
                for ki in range(KI):
                    nc.tensor.matmul(
                        pq[:],
                        w_sb["wq"][:, ki, eo * P:(eo + 1) * P],
                        gT[:, ki, :],
                        start=(ki == 0), stop=(ki == KI - 1))
                nc.scalar.copy(qT[:, eo, :], pq[:])
            return qT

        def emit_S_exp(j_in_chunk, qT):
            """S matmuls + exp/accum for one m-subtile; returns softmax state."""
            j = j_in_chunk
            s1 = ps_s.tile([P, NS], F32, tag="s")
            s2 = ps_s.tile([P, NS], F32, tag="s")
            for hk in range(2):
                for nch in range(NS // 512):
                    nc.tensor.matmul(
                        s1[:, nch * 512:(nch + 1) * 512],
                        qT[:, hk, j * P:(j + 1) * P],
                        kT_sb[:, hk, nch * 512:(nch + 1) * 512],
                        start=(hk == 0), stop=(hk == 1))
            p1 = ppool.tile([P, NS], F32, tag="p1")
            d1 = dpool.tile([P, 1], F32, tag="d")
            nc.scalar.activation(out=p1[:], in_=s1[:], func=AF.Exp,
                                 accum_out=d1[:])
            for hk in range(2):
                for nch in range(NS // 512):
                    nc.tensor.matmul(
                        s2[:, nch * 512:(nch + 1) * 512],
                        qT[:, 2 + hk, j * P:(j + 1) * P],
                        kT_sb[:, 2 + hk, nch * 512:(nch + 1) * 512],
                        start=(hk == 0), stop=(hk == 1))
            p2 = ppool.tile([P, NS], F32, tag="p2")
            d2 = dpool.tile([P, 1], F32, tag="d")
            nc.scalar.activation(out=p2[:], in_=s2[:], func=AF.Exp,
                                 accum_out=d2[:])
            return p1, d1, p2, d2

        def emit_softmax_tail(st, m0):
            """reciprocals + combine into diff; DMA diff out."""
            p1, d1, p2, d2 = st
            r1 = dpool.tile([P, 1], F32, tag="d")
            nc.vector.reciprocal(r1[:], d1[:])
            r2 = dpool.tile([P, 1], F32, tag="d")
            nc.vector.reciprocal(r2[:], d2[:])
            nc.vector.tensor_scalar(out=p2[:], in0=p2[:], scalar1=r2[:],
                                    scalar2=lam_sb[:], op0=ALU.mult,
                                    op1=ALU.mult)
            diff = dfpool.tile([P, NS], F32R, tag="diff")
            nc.vector.scalar_tensor_tensor(
                out=diff[:], in0=p1[:], scalar=r1[:], in1=p2[:],
                op0=ALU.mult, op1=ALU.subtract)
            nc.sync.dma_start(out=diff_d[m0:m0 + P, :], in_=diff[:])
            return diff

        def emit_transposes(diff):
            """diff -> diffT via PE transposes, copy to SBUF."""
            dT = dTpool.tile([P, NS], F32R, tag="dT")
            for half in range(2):
                tt = ps_u.tile([P, 512], F32R, tag="u")
                for nbh in range(4):
                    nb = half * 4 + nbh
                    nc.tensor.transpose(tt[:, nbh * P:(nbh + 1) * P],
                                        diff[:, nb * P:(nb + 1) * P],
                                        ident[:])
                nc.vector.tensor_copy(dT[:, half * 512:(half + 1) * 512],
                                      tt[:])
            return dT

        def emit_back(dT, m0):
            """PV + RMS + Wo projection + store for one m-subtile."""
            po = ps_u.tile([P, E], F32, tag="u")
            for nb in range(NB):
                nc.tensor.matmul(
                    po[:],
                    dT[:, nb * P:(nb + 1) * P],
                    v_sb[:, nb, :],
                    start=(nb == 0), stop=(nb == NB - 1))

            # RMS statistic (squares + row-sum in one DVE pass)
            o_sb = nopool.tile([P, E], F32R, tag="no")
            nc.scalar.copy(o_sb[:], po[:])
            sq = sqpool.tile([P, E], F32, tag="sq")
            ssq = dpool.tile([P, 1], F32, tag="d")
            nc.vector.scalar_tensor_tensor(
                out=sq[:], in0=o_sb[:], scalar=1.0, in1=o_sb[:],
                op0=ALU.mult, op1=ALU.mult, accum_out=ssq[:])
            t_ssq = dpool.tile([P, 1], F32, tag="d")
            nc.vector.tensor_scalar(out=t_ssq[:], in0=ssq[:],
                                    scalar1=1.0 / E, scalar2=RMS_EPS,
                                    op0=ALU.mult, op1=ALU.add)
            # rstd = 1/sqrt(t): Quake bit-trick + 2 Newton iterations on DVE
            I32 = mybir.dt.int32
            ihalf = dpool.tile([P, 1], F32, tag="d")
            nc.vector.tensor_scalar(
                out=ihalf[:].bitcast(I32), in0=t_ssq[:].bitcast(I32),
                scalar1=1, scalar2=None, op0=ALU.arith_shift_right)
            inot = dpool.tile([P, 1], F32, tag="d")
            nc.vector.tensor_scalar(
                out=inot[:].bitcast(I32), in0=ihalf[:].bitcast(I32),
                scalar1=-1, scalar2=None, op0=ALU.bitwise_xor)
            yq = dpool.tile([P, 1], F32, tag="d")
            nc.vector.tensor_scalar(
                out=yq[:].bitcast(I32), in0=inot[:].bitcast(I32),
                scalar1=0x5f3759df + 1, scalar2=None, op0=ALU.add)
            rstd = yq
            for _ in range(2):
                y2 = dpool.tile([P, 1], F32, tag="d")
                nc.vector.tensor_mul(y2[:], rstd[:], rstd[:])
                w = dpool.tile([P, 1], F32, tag="d")
                nc.vector.tensor_scalar(out=w[:], in0=y2[:],
                                        scalar1=t_ssq[:], scalar2=-0.5,
                                        op0=ALU.mult, op1=ALU.mult)
                yn = dpool.tile([P, 1], F32, tag="d")
                nc.vector.scalar_tensor_tensor(
                    out=yn[:], in0=w[:], scalar=1.5, in1=rstd[:],
                    op0=ALU.add, op1=ALU.mult)
                rstd = yn

            # transpose O, project with Wo, scale by rstd at the end
            nt = ps_u.tile([P, E], F32R, tag="u")
            for eb in range(EO):
                nc.tensor.transpose(nt[:, eb * P:(eb + 1) * P],
                                    o_sb[:, eb * P:(eb + 1) * P], ident[:])
            noT = nopool.tile([P, E], F32R, tag="noT")
            nc.scalar.copy(noT[:], nt[:])
            py = ps_u.tile([P, E], F32, tag="u")
            for eb in range(EO):
                nc.tensor.matmul(
                    py[:],
                    noT[:, eb * P:(eb + 1) * P],
                    w_sb["wo"][:, eb, :],
                    start=(eb == 0), stop=(eb == EO - 1))
            y = ypool.tile([P, E], F32, tag="y")
            nc.scalar.activation(out=y[:], in_=py[:], func=AF.Copy,
                                 scale=rstd[:])
            nc.sync.dma_start(out=out_d[m0:m0 + P, :], in_=y[:])

        # Software-pipelined emission with a 1-subtile skew: the PE stream is
        # S(j+1) -> transposes(j) -> PV(j) -> nt(j) -> y(j), so by the time
        # the PE reaches transposes(j), diff(j) (exp+combine on ACT/DVE) has
        # had a full subtile of slack to land.
        NTOT = NCH * JT
        qT = produce_qT(0)
        cur_qT, next_qT = qT, None
        prev = None                # (dT, m0) awaiting back-half
        prev_st = None             # softmax state awaiting tail
        prev_diff = None
        for idx in range(NTOT):
            c, j = divmod(idx, JT)
            if j == 0 and c > 0:
                cur_qT = next_qT
            st = emit_S_exp(j, cur_qT)
            if prev_diff is not None:
                dT = emit_transposes(prev_diff)
            diff = emit_softmax_tail(st, idx * P)
            if prev_diff is not None:
                emit_back(dT, (idx - 1) * P)
            if j == 1 and c + 1 < NCH:
                next_qT = produce_qT(c + 1)
            prev_diff = diff
        dT = emit_transposes(prev_diff)
        emit_back(dT, (NTOT - 1) * P)


# ---------------------------------------------------------------------------
_NC = None


def get_nc():
    global _NC
    if _NC is None:
        _NC = build_kernel()
    return _NC


def stage_inputs(gene, substructure, Wq, Wk, Wv, Wo,
                 lambda_q1, lambda_k1, lambda_q2, lambda_k2, rms_weight):
    gene = np.asarray(gene, np.float32)
    substructure = np.asarray(substructure, np.float32)
    scaling = H ** -0.5
    lam_full = (math.exp(float(np.sum(np.asarray(lambda_q1, np.float64) *
                                      np.asarray(lambda_k1, np.float64))))
                - math.exp(float(np.sum(np.asarray(lambda_q2, np.float64) *
                                        np.asarray(lambda_k2, np.float64))))
                + LAMBDA_INIT)
    wqT = np.ascontiguousarray(np.asarray(Wq, np.float32).T * scaling)
    wkT = np.ascontiguousarray(np.asarray(Wk, np.float32).T)
    wvT = np.ascontiguousarray(np.asarray(Wv, np.float32).T)
    woT = np.ascontiguousarray(
        np.asarray(rms_weight, np.float32)[:, None]
        * np.asarray(Wo, np.float32).T * (1.0 - LAMBDA_INIT))
    lam_tile = np.full((P, 1), lam_full, np.float32)
    in_maps = []
    for b in range(N_CORES):
        in_maps.append({
            "geneT": np.ascontiguousarray(gene[b].T),
            "subT": np.ascontiguousarray(substructure[b].T),
            "wqT": wqT, "wkT": wkT, "wvT": wvT, "woT": woT,
            "lam": lam_tile,
        })
    return in_maps


def run(in_maps, trace=False, **kw):
    nc = get_nc()
    last_err = None
    for attempt in range(3):
        try:
            return bass_utils.run_bass_kernel_spmd(
                nc, in_maps, core_ids=list(range(N_CORES)), trace=trace, **kw)
        except Exception as e:  # transient device errors on first touch
            last_err = e
    raise last_err


def kernel(**inputs):
    in_maps = stage_inputs(**inputs)
    res = run(in_maps, trace=False)
    out = np.stack([res.results[b]["out"] for b in range(N_CORES)])
    diff = np.stack([res.results[b]["diff"] for b in range(N_CORES)])
    return out, diff
